# revision 11
# baseline (speedup 1.0000x reference)
"""LSTM (SEQ=256, B=64, H=1024) on 8 Trainium2 NeuronCores.

Strategy
--------
Phase 1 (split 8-way over seq*batch): x_proj = x @ W_ih'^T + b' as a big GEMM,
  each core computes 2048 of the 16384 (s,b) rows; one AllGather replicates the
  full x_proj [16384, 4096] (bf16) to every core's HBM.
Phase 2 (replicated): every core runs the full recurrence. Cross-core
  collectives cost ~12us/call on this fabric and do not pipeline, so
  replicating the 268 MMAC/step recurrence (~7us on the PE with column-tiled
  concurrent matmuls) beats any per-step exchange scheme.

Algebra: sigmoid is folded into tanh by scaling i,f,o gate rows of the weights
and bias by 0.5 (sigmoid(z) = (tanh(z/2)+1)/2), and the cell/hidden state is
kept doubled (s=2c, ht=2h) so the whole cell update is 4 fused
scalar_tensor_tensor ops + 2 tanh activations per step:
  ti,tf,tg,to = tanh(gates')           # gates' pre-scaled via weights
  A = (tf+1)*s ; Bv = (ti+1)*tg ; s' = 0.5*A + Bv ; ht' = (to+1)*tanh(0.5*s')
Final h = ht/2 applied on host.

Matmul layout (phase 2): out chunk-pairs via tile_position column tiling:
  even gate-chunk -> PE columns 0:63 -> PSUM partitions 0:63
  odd  gate-chunk -> PE columns 64:127 -> PSUM partitions 64:127
Gates land as [128=(q,b), 2048=(type, u')] with gate g=1024*t+u at
  partition (u>=512)*64+b, free 512*t+u%512  -- full-width ACT/DVE ops.
"""
import numpy as np
from ml_dtypes import bfloat16

import concourse.bass as bass
import concourse.bacc as bacc
import concourse.mybir as mybir
from concourse import tile
from concourse.bass_utils import run_bass_kernel_spmd

S, B, H = 256, 64, 1024
G = 4 * H  # 4096 gates
NC = 8
SB = S * B  # 16384
SB_CORE = SB // NC  # 2048 rows of x_proj per core in phase 1
ST_TILES = SB_CORE // 128  # 16 sb-tiles per core in phase 1
KT = H // 128  # 8 contraction tiles
NCHUNK = G // 512  # 8 gate chunks of 512

F32 = mybir.dt.float32
BF16 = mybir.dt.bfloat16
Tanh = mybir.ActivationFunctionType.Tanh
Copy = mybir.ActivationFunctionType.Copy
ADD = mybir.AluOpType.add
MULT = mybir.AluOpType.mult


def build(n_steps=S):
    nc = bacc.Bacc(None, target_bir_lowering=False)

    # ---- external inputs (per core) ----
    xT = nc.dram_tensor("xT", [H, SB_CORE], BF16, kind="ExternalInput")
    wihT = nc.dram_tensor("wihT", [H, G], BF16, kind="ExternalInput")
    whhT = nc.dram_tensor("whhT", [H, G], BF16, kind="ExternalInput")
    bias = nc.dram_tensor("bias", [1, G], BF16, kind="ExternalInput")
    # identity stacked twice so both partition halves have a copy at their base
    ident = nc.dram_tensor("ident", [128, 64], BF16, kind="ExternalInput")
    ones = nc.dram_tensor("ones", [1, 128], BF16, kind="ExternalInput")
    h_out = nc.dram_tensor("h_last", [B, H], F32, kind="ExternalOutput")

    # ---- internal DRAM ----
    cin = nc.dram_tensor("cin", [SB_CORE, G], BF16)
    cout = nc.dram_tensor("cout", [SB, G], BF16, addr_space="Shared")

    with tile.TileContext(nc) as tc:
        # ================= Phase 1: x_proj chunk =================
        with (
            tc.tile_pool(name="wih", bufs=1) as wih_pool,
            tc.tile_pool(name="cst", bufs=1) as cst_pool,
            tc.tile_pool(name="xts", bufs=3) as xts_pool,
            tc.tile_pool(name="xpsb", bufs=3) as xpsb_pool,
            tc.tile_pool(name="ps1", bufs=1, space="PSUM") as ps1_pool,
        ):
            wih_sb = wih_pool.tile([128, KT * G], BF16)  # [p, (k,g)]
            nc.sync.dma_start(
                out=wih_sb[:].rearrange("p (k g) -> p k g", k=KT),
                in_=wihT.ap().rearrange("(k p) g -> p k g", p=128),
            )
            bias_sb = cst_pool.tile([1, G], BF16, tag="bias")
            nc.sync.dma_start(out=bias_sb[:], in_=bias.ap())
            ones_sb = cst_pool.tile([1, 128], BF16, tag="ones")
            nc.sync.dma_start(out=ones_sb[:], in_=ones.ap())

            for st in range(ST_TILES):
                xts = xts_pool.tile([128, KT * 128], BF16)  # [sb-part? no: [p=k-rows, (k, sb128)]
                # xT slice [1024, 128] -> 8 k-tiles [128 h, 128 sb]
                nc.sync.dma_start(
                    out=xts[:].rearrange("p (k s) -> p k s", k=KT),
                    in_=xT.ap()[:, st * 128:(st + 1) * 128].rearrange(
                        "(k p) s -> p k s", p=128
                    ),
                )
                psums = [ps1_pool.tile([128, 512], F32, tag=f"b{n}", name=f"ps1_{n}") for n in range(NCHUNK)]
                for n in range(NCHUNK):
                    # bias via ones-row outer product; clears the bank
                    nc.tensor.matmul(
                        psums[n][:],
                        lhsT=ones_sb[:],
                        rhs=bias_sb[:, n * 512:(n + 1) * 512],
                        start=True,
                        stop=False,
                    )
                for k in range(KT):
                    for n in range(NCHUNK):
                        nc.tensor.matmul(
                            psums[n][:],
                            lhsT=xts[:, k * 128:(k + 1) * 128],
                            rhs=wih_sb[:, k * G + n * 512: k * G + (n + 1) * 512],
                            start=False,
                            stop=(k == KT - 1),
                        )
                xp_sb = xpsb_pool.tile([128, G], BF16)
                for n in range(NCHUNK):
                    nc.scalar.copy(out=xp_sb[:, n * 512:(n + 1) * 512], in_=psums[n][:])
                nc.sync.dma_start(
                    out=cin.ap()[st * 128:(st + 1) * 128, :], in_=xp_sb[:]
                )

        # ================= AllGather x_proj =================
        nc.gpsimd.collective_compute(
            "AllGather",
            mybir.AluOpType.bypass,
            ins=[cin.ap().opt()],
            outs=[cout.ap().opt()],
            replica_groups=[list(range(NC))],
        )

        # ================= Phase 2: recurrence =================
        with (
            tc.tile_pool(name="whh", bufs=1) as whh_pool,
            tc.tile_pool(name="cst2", bufs=1) as cst2_pool,
            tc.tile_pool(name="state", bufs=1) as state_pool,
            tc.tile_pool(name="xpt", bufs=3) as xpt_pool,
            tc.tile_pool(name="work", bufs=2) as work_pool,
            tc.tile_pool(name="ps2", bufs=1, space="PSUM") as ps2_pool,
            tc.tile_pool(name="pstr", bufs=1, space="PSUM") as pstr_pool,
        ):
            whh_sb = whh_pool.tile([128, KT * G], BF16)
            nc.sync.dma_start(
                out=whh_sb[:].rearrange("p (k g) -> p k g", k=KT),
                in_=whhT.ap().rearrange("(k p) g -> p k g", p=128),
            )
            id_sb = cst2_pool.tile([128, 64], BF16, tag="id")
            nc.sync.dma_start(out=id_sb[:], in_=ident.ap())

            # state: ping-pong ht^T (stationary tiles) and s=2c
            hts = [state_pool.tile([128, KT * 64], BF16, tag=f"ht{i}", name=f"ht_{i}") for i in range(2)]
            ss = [state_pool.tile([128, 512], F32, tag=f"s{i}", name=f"s_{i}") for i in range(2)]
            nc.vector.memset(hts[0][:], 0)
            nc.vector.memset(ss[0][:], 0)

            for t in range(n_steps):
                ht_cur = hts[t % 2]
                ht_nxt = hts[(t + 1) % 2]
                s_cur = ss[t % 2]
                s_nxt = ss[(t + 1) % 2]

                # x_proj rows for this step: [64, 4096] -> [128=(q,b), 2048]
                xpt = xpt_pool.tile([128, 2048], BF16)
                src = cout.ap()[t * 64:(t + 1) * 64, :].rearrange(
                    "b (ty q u) -> q b ty u", q=2, u=512
                )
                for q in range(2):
                    nc.sync.dma_start(
                        out=xpt[q * 64:(q + 1) * 64, :].rearrange(
                            "b (ty u) -> b ty u", u=512
                        ),
                        in_=src[q],
                    )

                # gate matmuls: 4 PSUM banks, chunk-pair per bank via col tiling
                gps = [ps2_pool.tile([128, 512], F32, tag=f"g{ty}", name=f"gps_{ty}") for ty in range(4)]
                for k in range(KT):
                    hk = ht_cur[:, k * 64:(k + 1) * 64]
                    for ty in range(4):
                        # even chunk: gates [1024*ty : 1024*ty+512] -> partitions 0:64
                        nc.tensor.matmul(
                            gps[ty][0:64, :],
                            lhsT=hk,
                            rhs=whh_sb[:, k * G + ty * 1024: k * G + ty * 1024 + 512],
                            start=(k == 0),
                            stop=(k == KT - 1),
                            tile_position=(0, 0),
                        )
                        # odd chunk -> partitions 64:128
                        nc.tensor.matmul(
                            gps[ty][64:128, :],
                            lhsT=hk,
                            rhs=whh_sb[:, k * G + ty * 1024 + 512: k * G + (ty + 1) * 1024],
                            start=(k == 0),
                            stop=(k == KT - 1),
                            tile_position=(0, 64),
                        )

                # add x_proj: gsb = psum + xpt   [128, 2048] fp32
                gsb = work_pool.tile([128, 2048], F32, tag="gsb")
                for ty in range(4):
                    nc.vector.scalar_tensor_tensor(
                        out=gsb[:, ty * 512:(ty + 1) * 512],
                        in0=gps[ty][:],
                        scalar=0.0,
                        in1=xpt[:, ty * 512:(ty + 1) * 512],
                        op0=ADD,
                        op1=ADD,
                    )

                # activations: ti,tf,tg then to
                t3 = work_pool.tile([128, 1536], F32, tag="t3")
                nc.scalar.activation(t3[:], gsb[:, 0:1536], Tanh)
                t_o = work_pool.tile([128, 512], F32, tag="to")
                nc.scalar.activation(t_o[:], gsb[:, 1536:2048], Tanh)

                # cell update
                av = work_pool.tile([128, 512], F32, tag="av")
                bv = work_pool.tile([128, 512], F32, tag="bv")
                nc.vector.scalar_tensor_tensor(
                    out=av[:], in0=t3[:, 512:1024], scalar=1.0, in1=s_cur[:],
                    op0=ADD, op1=MULT,
                )
                nc.vector.scalar_tensor_tensor(
                    out=bv[:], in0=t3[:, 0:512], scalar=1.0, in1=t3[:, 1024:1536],
                    op0=ADD, op1=MULT,
                )
                nc.vector.scalar_tensor_tensor(
                    out=s_nxt[:], in0=av[:], scalar=0.5, in1=bv[:],
                    op0=MULT, op1=ADD,
                )
                tc_t = work_pool.tile([128, 512], F32, tag="tc")
                nc.scalar.activation(tc_t[:], s_nxt[:], Tanh, scale=0.5)

                if t < n_steps - 1:
                    # ht' = (to+1)*tc, chunked by k-tile, bf16, then transpose
                    htn = work_pool.tile([128, 512], BF16, tag="htn")
                    nc.vector.scalar_tensor_tensor(
                        out=htn[:], in0=t_o[:], scalar=1.0, in1=tc_t[:],
                        op0=ADD, op1=MULT,
                    )
                    # htn [128=(q,b), 512=u'] holds ht for h-unit 512q+u', batch b.
                    # k-tile k covers h [128k:128k+128): q=k//4, u' in [128*(k%4)...)
                    for k in range(KT):
                        q, blk = k // 4, k % 4
                        pst = pstr_pool.tile([128, 64], BF16, tag=f"tr{k % 4}")
                        nc.tensor.transpose(
                            pst[:],
                            htn[q * 64:(q + 1) * 64, blk * 128:(blk + 1) * 128],
                            id_sb[q * 64:(q + 1) * 64, :],
                        )
                        nc.vector.tensor_copy(
                            ht_nxt[:, k * 64:(k + 1) * 64], pst[:]
                        )
                else:
                    # final step: h_tilde = (to+1)*tc in fp32, reshape to [64, 1024]
                    hfin = work_pool.tile([128, 512], F32, tag="hfin")
                    nc.vector.scalar_tensor_tensor(
                        out=hfin[:], in0=t_o[:], scalar=1.0, in1=tc_t[:],
                        op0=ADD, op1=MULT,
                    )
                    # hfin [(q,b), u'] -> h_out[b, 512q+u']
                    for q in range(2):
                        nc.sync.dma_start(
                            out=h_out.ap()[:, q * 512:(q + 1) * 512],
                            in_=hfin[q * 64:(q + 1) * 64, :],
                        )

    nc.compile()
    return nc


_CACHED = {}


def _get_nc(n_steps=S):
    if n_steps not in _CACHED:
        _CACHED[n_steps] = build(n_steps)
    return _CACHED[n_steps]


def prepare_inputs(x, W_ih, W_hh, b_ih, b_hh):
    """Host-side layout prep. Returns per-core input maps."""
    x = np.asarray(x, dtype=np.float32)
    W_ih = np.asarray(W_ih, dtype=np.float32)
    W_hh = np.asarray(W_hh, dtype=np.float32)
    b = np.asarray(b_ih, dtype=np.float32) + np.asarray(b_hh, dtype=np.float32)

    # gate-row scaling: i,f,o rows *0.5 (sigmoid fold); W_hh all rows *0.5 (ht=2h)
    col = np.ones(G, dtype=np.float32)
    col[0:H] = 0.5          # i
    col[H:2 * H] = 0.5      # f
    col[3 * H:4 * H] = 0.5  # o
    Wih_s = W_ih * col[:, None]
    Whh_s = W_hh * (0.5 * col[:, None])
    b_s = b * col

    wihT = np.ascontiguousarray(Wih_s.T).astype(bfloat16)  # [H, G]
    whhT = np.ascontiguousarray(Whh_s.T).astype(bfloat16)
    bias = b_s.reshape(1, G).astype(bfloat16)
    xT_full = np.ascontiguousarray(x.reshape(SB, H).T).astype(bfloat16)  # [H, SB]
    eye64 = np.eye(64, dtype=np.float32)
    ident = np.vstack([eye64, eye64]).astype(bfloat16)  # [128, 64]
    ones = np.ones((1, 128), dtype=np.float32).astype(bfloat16)

    in_maps = []
    for j in range(NC):
        in_maps.append({
            "xT": np.ascontiguousarray(xT_full[:, j * SB_CORE:(j + 1) * SB_CORE]),
            "wihT": wihT,
            "whhT": whhT,
            "bias": bias,
            "ident": ident,
            "ones": ones,
        })
    return in_maps


def kernel(x, W_ih, W_hh, b_ih, b_hh, trace=False, n_steps=S):
    nc = _get_nc(n_steps)
    in_maps = prepare_inputs(x, W_ih, W_hh, b_ih, b_hh)
    res = run_bass_kernel_spmd(nc, in_maps, core_ids=list(range(NC)), trace=trace)
    h_tilde = res.results[0]["h_last"]
    out = (0.5 * h_tilde).astype(np.float32)
    if trace:
        kernel.last_exec_time_ns = res.exec_time_ns
    return out


# revision 17
# speedup vs baseline: 1.2313x; 1.2313x over previous
"""LSTM (SEQ=256, B=64, H=1024) on 8 Trainium2 NeuronCores.

Strategy
--------
Phase 1 (split 8-way over seq*batch): x_proj = x @ W_ih'^T + b' as a big GEMM,
  each core computes 2048 of the 16384 (s,b) rows; one AllGather replicates the
  full x_proj [16384, 4096] (bf16) to every core's HBM.
Phase 2 (replicated): every core runs the full recurrence. Cross-core
  collectives cost ~12us/call on this fabric and do not pipeline, so
  replicating the 268 MMAC/step recurrence (~7us on the PE with column-tiled
  concurrent matmuls) beats any per-step exchange scheme.

Algebra: sigmoid is folded into tanh by scaling i,f,o gate rows of the weights
and bias by 0.5 (sigmoid(z) = (tanh(z/2)+1)/2), and the cell/hidden state is
kept doubled (s=2c, ht=2h) so the whole cell update is 4 fused
scalar_tensor_tensor ops + 2 tanh activations per step:
  ti,tf,tg,to = tanh(gates')           # gates' pre-scaled via weights
  A = (tf+1)*s ; Bv = (ti+1)*tg ; s' = 0.5*A + Bv ; ht' = (to+1)*tanh(0.5*s')
Final h = ht/2 applied on host.

Matmul layout (phase 2): out chunk-pairs via tile_position column tiling:
  even gate-chunk -> PE columns 0:63 -> PSUM partitions 0:63
  odd  gate-chunk -> PE columns 64:127 -> PSUM partitions 64:127
Gates land as [128=(q,b), 2048=(type, u')] with gate g=1024*t+u at
  partition (u>=512)*64+b, free 512*t+u%512  -- full-width ACT/DVE ops.
"""
import numpy as np
from ml_dtypes import bfloat16

import concourse.bass as bass
import concourse.bacc as bacc
import concourse.mybir as mybir
from concourse import tile
from concourse.bass_utils import run_bass_kernel_spmd

S, B, H = 256, 64, 1024
G = 4 * H  # 4096 gates
NC = 8
SB = S * B  # 16384
SB_CORE = SB // NC  # 2048 rows of x_proj per core in phase 1
ST_TILES = SB_CORE // 128  # 16 sb-tiles per core in phase 1
KT = H // 128  # 8 contraction tiles
NCHUNK = G // 512  # 8 gate chunks of 512

F32 = mybir.dt.float32
BF16 = mybir.dt.bfloat16
Tanh = mybir.ActivationFunctionType.Tanh
Copy = mybir.ActivationFunctionType.Copy
ADD = mybir.AluOpType.add
MULT = mybir.AluOpType.mult


def build(n_steps=S):
    nc = bacc.Bacc(None, target_bir_lowering=False)

    # ---- external inputs (per core) ----
    xT = nc.dram_tensor("xT", [H, SB_CORE], BF16, kind="ExternalInput")
    wihT = nc.dram_tensor("wihT", [H, G], BF16, kind="ExternalInput")
    whhT = nc.dram_tensor("whhT", [H, G], BF16, kind="ExternalInput")
    bias = nc.dram_tensor("bias", [1, G], BF16, kind="ExternalInput")
    # identity stacked twice so both partition halves have a copy at their base
    ident = nc.dram_tensor("ident", [128, 64], BF16, kind="ExternalInput")
    id128 = nc.dram_tensor("id128", [128, 128], BF16, kind="ExternalInput")
    ones = nc.dram_tensor("ones", [1, 128], BF16, kind="ExternalInput")
    h_out = nc.dram_tensor("h_last", [B, H], F32, kind="ExternalOutput")

    # ---- internal DRAM ----
    cin = nc.dram_tensor("cin", [SB_CORE, G], BF16)
    cout = nc.dram_tensor("cout", [SB, G], BF16, addr_space="Shared")

    with tile.TileContext(nc) as tc:
        # ================= Phase 1: x_proj chunk =================
        with (
            tc.tile_pool(name="wih", bufs=1) as wih_pool,
            tc.tile_pool(name="cst", bufs=1) as cst_pool,
            tc.tile_pool(name="xts", bufs=3) as xts_pool,
            tc.tile_pool(name="xpsb", bufs=3) as xpsb_pool,
            tc.tile_pool(name="ps1", bufs=2, space="PSUM") as ps1_pool,
        ):
            wih_sb = wih_pool.tile([128, KT * G], BF16)  # [p, (k,g)]
            nc.sync.dma_start(
                out=wih_sb[:].rearrange("p (k g) -> p k g", k=KT),
                in_=wihT.ap().rearrange("(k p) g -> p k g", p=128),
            )
            bias_sb = cst_pool.tile([1, G], BF16, tag="bias")
            nc.sync.dma_start(out=bias_sb[:], in_=bias.ap())
            ones_sb = cst_pool.tile([1, 128], BF16, tag="ones")
            nc.sync.dma_start(out=ones_sb[:], in_=ones.ap())

            for st in range(ST_TILES):
                xts = xts_pool.tile([128, KT * 128], BF16)  # [sb-part? no: [p=k-rows, (k, sb128)]
                # xT slice [1024, 128] -> 8 k-tiles [128 h, 128 sb]
                nc.sync.dma_start(
                    out=xts[:].rearrange("p (k s) -> p k s", k=KT),
                    in_=xT.ap()[:, st * 128:(st + 1) * 128].rearrange(
                        "(k p) s -> p k s", p=128
                    ),
                )
                xp_sb = xpsb_pool.tile([128, G], BF16)
                # n-outer, k-inner: each bank completes then evacuates while the
                # next bank's matmuls run -> PSUM frees progressively, PE stays warm
                for n in range(NCHUNK):
                    psum = ps1_pool.tile([128, 512], F32, tag=f"b{n % 2}", name=f"ps1_{n}")
                    # bias via ones-row outer product; clears the bank
                    nc.tensor.matmul(
                        psum[:],
                        lhsT=ones_sb[:],
                        rhs=bias_sb[:, n * 512:(n + 1) * 512],
                        start=True,
                        stop=False,
                    )
                    for k in range(KT):
                        nc.tensor.matmul(
                            psum[:],
                            lhsT=xts[:, k * 128:(k + 1) * 128],
                            rhs=wih_sb[:, k * G + n * 512: k * G + (n + 1) * 512],
                            start=False,
                            stop=(k == KT - 1),
                        )
                    nc.scalar.copy(out=xp_sb[:, n * 512:(n + 1) * 512], in_=psum[:])
                nc.sync.dma_start(
                    out=cin.ap()[st * 128:(st + 1) * 128, :], in_=xp_sb[:]
                )

        # ================= AllGather x_proj =================
        nc.gpsimd.collective_compute(
            "AllGather",
            mybir.AluOpType.bypass,
            ins=[cin.ap().opt()],
            outs=[cout.ap().opt()],
            replica_groups=[list(range(NC))],
        )

        # ================= Phase 2: recurrence =================
        with (
            tc.tile_pool(name="whh", bufs=1) as whh_pool,
            tc.tile_pool(name="cst2", bufs=1) as cst2_pool,
            tc.tile_pool(name="state", bufs=1) as state_pool,
            tc.tile_pool(name="xpt", bufs=3) as xpt_pool,
            tc.tile_pool(name="work", bufs=2) as work_pool,
            tc.tile_pool(name="ps2", bufs=1, space="PSUM") as ps2_pool,
            tc.tile_pool(name="pstr", bufs=1, space="PSUM") as pstr_pool,
        ):
            whh_sb = whh_pool.tile([128, KT * G], BF16)
            nc.sync.dma_start(
                out=whh_sb[:].rearrange("p (k g) -> p k g", k=KT),
                in_=whhT.ap().rearrange("(k p) g -> p k g", p=128),
            )
            id_sb = cst2_pool.tile([128, 64], BF16, tag="id")
            nc.sync.dma_start(out=id_sb[:], in_=ident.ap())
            id128_sb = cst2_pool.tile([128, 128], BF16, tag="id128")
            nc.sync.dma_start(out=id128_sb[:], in_=id128.ap())

            # state: ping-pong ht^T (stationary tiles); chain = [ti,tf,tg,s]
            hts = [state_pool.tile([128, KT * 64], BF16, tag=f"ht{i}", name=f"ht_{i}") for i in range(2)]
            chain = state_pool.tile([128, 2048], F32, tag="chain")
            nc.vector.memset(hts[0][:], 0)
            nc.vector.memset(chain[:, 1536:2048], 0)  # s = 2c = 0

            for t in range(n_steps):
                ht_cur = hts[t % 2]
                ht_nxt = hts[(t + 1) % 2]

                # x_proj rows for this step: [64, 4096] -> [128=(q,b), 2048]
                xpt = xpt_pool.tile([128, 2048], BF16)
                src = cout.ap()[t * 64:(t + 1) * 64, :].rearrange(
                    "b (ty q u) -> q b ty u", q=2, u=512
                )
                for q in range(2):
                    nc.sync.dma_start(
                        out=xpt[q * 64:(q + 1) * 64, :].rearrange(
                            "b (ty u) -> b ty u", u=512
                        ),
                        in_=src[q],
                    )

                # gate matmuls, ty-outer so each bank finishes early and its tanh
                # + cell update hide under the later banks' matmuls.
                # Bank ty: identity-copy of x_proj (start=True) then 8 k-rounds of
                # column-tiled concurrent pairs.
                gps = [ps2_pool.tile([128, 512], F32, tag=f"g{ty}", name=f"gps_{ty}") for ty in range(4)]
                t_o = work_pool.tile([128, 512], F32, tag="to")
                for ty in range(4):
                    nc.tensor.matmul(
                        gps[ty][:],
                        lhsT=id128_sb[:],
                        rhs=xpt[:, ty * 512:(ty + 1) * 512],
                        start=True,
                        stop=False,
                    )
                    for k in range(KT):
                        hk = ht_cur[:, k * 64:(k + 1) * 64]
                        nc.tensor.matmul(
                            gps[ty][0:64, :],
                            lhsT=hk,
                            rhs=whh_sb[:, k * G + ty * 1024: k * G + ty * 1024 + 512],
                            start=False,
                            stop=False,
                            tile_position=(0, 0),
                        )
                        nc.tensor.matmul(
                            gps[ty][64:128, :],
                            lhsT=hk,
                            rhs=whh_sb[:, k * G + ty * 1024 + 512: k * G + (ty + 1) * 1024],
                            start=False,
                            stop=(k == KT - 1),
                            tile_position=(0, 64),
                        )
                    # tanh straight out of PSUM into the chain tile
                    if ty < 3:
                        nc.scalar.activation(
                            chain[:, ty * 512:(ty + 1) * 512], gps[ty][:], Tanh
                        )
                    else:
                        nc.scalar.activation(t_o[:], gps[ty][:], Tanh)

                # cell update (overlaps bank 3's matmuls):
                # AB[0:512] = (ti+1)*tg ; AB[512:1024] = (tf+1)*s
                ab = work_pool.tile([128, 1024], F32, tag="ab")
                nc.vector.scalar_tensor_tensor(
                    out=ab[:], in0=chain[:, 0:1024], scalar=1.0,
                    in1=chain[:, 1024:2048], op0=ADD, op1=MULT,
                )
                nc.vector.scalar_tensor_tensor(
                    out=chain[:, 1536:2048], in0=ab[:, 512:1024], scalar=0.5,
                    in1=ab[:, 0:512], op0=MULT, op1=ADD,
                )
                tc_t = work_pool.tile([128, 512], F32, tag="tc")
                nc.scalar.activation(tc_t[:], chain[:, 1536:2048], Tanh, scale=0.5)

                if t < n_steps - 1:
                    # ht' = (to+1)*tc per k-chunk, bf16, then transpose; chunked so
                    # transposes pipeline into the next step's k-rounds
                    htn = work_pool.tile([128, 512], BF16, tag="htn")
                    for k in range(KT):
                        q, blk = k // 4, k % 4
                        sl = slice(blk * 128, (blk + 1) * 128)
                        rows = slice(q * 64, (q + 1) * 64)
                        nc.vector.scalar_tensor_tensor(
                            out=htn[rows, sl], in0=t_o[rows, sl], scalar=1.0,
                            in1=tc_t[rows, sl], op0=ADD, op1=MULT,
                        )
                        pst = pstr_pool.tile([128, 64], BF16, tag=f"tr{k % 4}")
                        nc.tensor.transpose(
                            pst[:], htn[rows, sl], id_sb[rows, :]
                        )
                        nc.vector.tensor_copy(
                            ht_nxt[:, k * 64:(k + 1) * 64], pst[:]
                        )
                else:
                    # final step: h_tilde = (to+1)*tc in fp32, reshape to [64, 1024]
                    hfin = work_pool.tile([128, 512], F32, tag="hfin")
                    nc.vector.scalar_tensor_tensor(
                        out=hfin[:], in0=t_o[:], scalar=1.0, in1=tc_t[:],
                        op0=ADD, op1=MULT,
                    )
                    # hfin [(q,b), u'] -> h_out[b, 512q+u']
                    for q in range(2):
                        nc.sync.dma_start(
                            out=h_out.ap()[:, q * 512:(q + 1) * 512],
                            in_=hfin[q * 64:(q + 1) * 64, :],
                        )

    nc.compile()
    return nc


_CACHED = {}


def _get_nc(n_steps=S):
    if n_steps not in _CACHED:
        _CACHED[n_steps] = build(n_steps)
    return _CACHED[n_steps]


def prepare_inputs(x, W_ih, W_hh, b_ih, b_hh):
    """Host-side layout prep. Returns per-core input maps."""
    x = np.asarray(x, dtype=np.float32)
    W_ih = np.asarray(W_ih, dtype=np.float32)
    W_hh = np.asarray(W_hh, dtype=np.float32)
    b = np.asarray(b_ih, dtype=np.float32) + np.asarray(b_hh, dtype=np.float32)

    # gate-row scaling: i,f,o rows *0.5 (sigmoid fold); W_hh all rows *0.5 (ht=2h)
    col = np.ones(G, dtype=np.float32)
    col[0:H] = 0.5          # i
    col[H:2 * H] = 0.5      # f
    col[3 * H:4 * H] = 0.5  # o
    Wih_s = W_ih * col[:, None]
    Whh_s = W_hh * (0.5 * col[:, None])
    b_s = b * col

    wihT = np.ascontiguousarray(Wih_s.T).astype(bfloat16)  # [H, G]
    whhT = np.ascontiguousarray(Whh_s.T).astype(bfloat16)
    bias = b_s.reshape(1, G).astype(bfloat16)
    xT_full = np.ascontiguousarray(x.reshape(SB, H).T).astype(bfloat16)  # [H, SB]
    eye64 = np.eye(64, dtype=np.float32)
    ident = np.vstack([eye64, eye64]).astype(bfloat16)  # [128, 64]
    id128_np = np.eye(128, dtype=np.float32).astype(bfloat16)
    ones = np.ones((1, 128), dtype=np.float32).astype(bfloat16)

    in_maps = []
    for j in range(NC):
        in_maps.append({
            "xT": np.ascontiguousarray(xT_full[:, j * SB_CORE:(j + 1) * SB_CORE]),
            "wihT": wihT,
            "whhT": whhT,
            "bias": bias,
            "ident": ident,
            "id128": id128_np,
            "ones": ones,
        })
    return in_maps


def kernel(x, W_ih, W_hh, b_ih, b_hh, trace=False, n_steps=S):
    nc = _get_nc(n_steps)
    in_maps = prepare_inputs(x, W_ih, W_hh, b_ih, b_hh)
    res = run_bass_kernel_spmd(nc, in_maps, core_ids=list(range(NC)), trace=trace)
    h_tilde = res.results[0]["h_last"]
    out = (0.5 * h_tilde).astype(np.float32)
    if trace:
        kernel.last_exec_time_ns = res.exec_time_ns
    return out


# revision 20
# speedup vs baseline: 1.3779x; 1.1191x over previous
"""LSTM (SEQ=256, B=64, H=1024) on 8 Trainium2 NeuronCores.

Strategy
--------
Phase 1 (split 8-way over seq*batch): x_proj = x @ W_ih'^T + b' as a big GEMM;
  core j computes rows [2048j, 2048j+2048) = time steps [32j, 32j+32).
  The AllGather that replicates x_proj to every core is split into 8 sub-AGs
  (one per 4-step stripe of every core's block) so the gather pipelines behind
  phase 1 and the start of the recurrence.
Phase 2 (replicated): every core runs the full recurrence. Cross-core
  collectives cost ~12us/call on this fabric and do not pipeline, so
  replicating the 268 MMAC/step recurrence beats any per-step exchange.

Algebra: sigmoid folded into tanh via 0.5-scaled i,f,o weight rows
(sigmoid(z) = (tanh(z/2)+1)/2); state kept doubled (s=2c, ht=2h):
  ti,tf,tg,to = tanh(gates')          # pre-scaled via weights
  A=(tf+1)*s ; Bv=(ti+1)*tg ; s'=0.5*A+Bv ; ht'=(to+1)*tanh(0.5*s')
Final h = ht/2 on host.

Phase-2 matmul: chunk-pairs via tile_position column tiling (concurrent
matmuls in the two column halves, separate XBUS streams):
  even gate-chunk -> PE cols 0:63 -> PSUM partitions 0:63
  odd  gate-chunk -> PE cols 64:127 -> PSUM partitions 64:127
x_proj is injected into PSUM with an identity matmul (start=True), so the
whole gate pre-activation accumulates in PSUM and ACT reads it directly.
Per-gate-type tanh + the cell update hide under later banks' matmuls; the
h-transposes (PE transpose mode) pipeline into the next step's k-rounds.
"""
import numpy as np
from ml_dtypes import bfloat16

import concourse.bass as bass
import concourse.bacc as bacc
import concourse.mybir as mybir
from concourse import tile
from concourse.bass_utils import run_bass_kernel_spmd

S, B, H = 256, 64, 1024
G = 4 * H
NC = 8
SB = S * B  # 16384
SB_CORE = SB // NC  # 2048
ST_TILES = SB_CORE // 128  # 16
KT = H // 128  # 8
NCHUNK = G // 512  # 8
NSUB = 8  # sub-AllGathers
SUB_ROWS = SB_CORE // NSUB  # 256 rows per rank per sub-AG (4 steps)

F32 = mybir.dt.float32
BF16 = mybir.dt.bfloat16
Tanh = mybir.ActivationFunctionType.Tanh
ADD = mybir.AluOpType.add
MULT = mybir.AluOpType.mult


def cout_row(t):
    """Row offset in the permuted cout for step t (64 rows per step).

    cout layout: [sub p][rank j][256 rows = 4 steps * 64].
    Step t = 32j + 4p + rr lives at 2048p + 256j + 64rr.
    """
    j, r = divmod(t, 32)
    p, rr = divmod(r, 4)
    return 2048 * p + 256 * j + 64 * rr


def build(n_steps=S):
    nc = bacc.Bacc(None, target_bir_lowering=False)

    xT = nc.dram_tensor("xT", [H, SB_CORE], BF16, kind="ExternalInput")
    wihT = nc.dram_tensor("wihT", [H, G], BF16, kind="ExternalInput")
    whhT = nc.dram_tensor("whhT", [H, G], BF16, kind="ExternalInput")
    bias = nc.dram_tensor("bias", [1, G], BF16, kind="ExternalInput")
    ident = nc.dram_tensor("ident", [128, 64], BF16, kind="ExternalInput")
    id128 = nc.dram_tensor("id128", [128, 128], BF16, kind="ExternalInput")
    ones = nc.dram_tensor("ones", [1, 128], BF16, kind="ExternalInput")
    h_out = nc.dram_tensor("h_last", [B, H], F32, kind="ExternalOutput")

    cin = nc.dram_tensor("cin", [SB_CORE, G], BF16)
    cout = nc.dram_tensor("cout", [SB, G], BF16, addr_space="Shared")

    with tile.TileContext(nc) as tc:
        with (
            tc.tile_pool(name="wih", bufs=1) as wih_pool,
            tc.tile_pool(name="whh", bufs=1) as whh_pool,
            tc.tile_pool(name="cst", bufs=1) as cst_pool,
            tc.tile_pool(name="state", bufs=1) as state_pool,
            tc.tile_pool(name="xts", bufs=3) as xts_pool,
            tc.tile_pool(name="xpsb", bufs=2) as xpsb_pool,
            tc.tile_pool(name="xpt", bufs=3) as xpt_pool,
            tc.tile_pool(name="work", bufs=1) as work_pool,
            tc.tile_pool(name="ps1", bufs=1, space="PSUM") as ps1_pool,
            tc.tile_pool(name="ps2", bufs=1, space="PSUM") as ps2_pool,
            tc.tile_pool(name="pstr", bufs=1, space="PSUM") as pstr_pool,
        ):
            # ---- constants / weights ----
            wih_sb = wih_pool.tile([128, KT * G], BF16)
            nc.sync.dma_start(
                out=wih_sb[:].rearrange("p (k g) -> p k g", k=KT),
                in_=wihT.ap().rearrange("(k p) g -> p k g", p=128),
            )
            whh_sb = whh_pool.tile([128, KT * G], BF16)
            nc.sync.dma_start(
                out=whh_sb[:].rearrange("p (k g) -> p k g", k=KT),
                in_=whhT.ap().rearrange("(k p) g -> p k g", p=128),
            )
            bias_sb = cst_pool.tile([1, G], BF16, tag="bias")
            nc.sync.dma_start(out=bias_sb[:], in_=bias.ap())
            ones_sb = cst_pool.tile([1, 128], BF16, tag="ones")
            nc.sync.dma_start(out=ones_sb[:], in_=ones.ap())
            id_sb = cst_pool.tile([128, 64], BF16, tag="id")
            nc.sync.dma_start(out=id_sb[:], in_=ident.ap())
            id128_sb = cst_pool.tile([128, 128], BF16, tag="id128")
            nc.sync.dma_start(out=id128_sb[:], in_=id128.ap())

            # ---- phase 1: x_proj rows, st-tile at a time ----
            for st in range(ST_TILES):
                xts = xts_pool.tile([128, KT * 128], BF16)
                nc.sync.dma_start(
                    out=xts[:].rearrange("p (k s) -> p k s", k=KT),
                    in_=xT.ap()[:, st * 128:(st + 1) * 128].rearrange(
                        "(k p) s -> p k s", p=128
                    ),
                )
                xp_sb = xpsb_pool.tile([128, G], BF16)
                for n in range(NCHUNK):
                    psum = ps1_pool.tile(
                        [128, 512], F32, tag=f"b{n % 2}", name=f"ps1_{n}"
                    )
                    nc.tensor.matmul(
                        psum[:],
                        lhsT=ones_sb[:],
                        rhs=bias_sb[:, n * 512:(n + 1) * 512],
                        start=True,
                        stop=False,
                    )
                    for k in range(KT):
                        nc.tensor.matmul(
                            psum[:],
                            lhsT=xts[:, k * 128:(k + 1) * 128],
                            rhs=wih_sb[:, k * G + n * 512: k * G + (n + 1) * 512],
                            start=False,
                            stop=(k == KT - 1),
                        )
                    nc.scalar.copy(out=xp_sb[:, n * 512:(n + 1) * 512], in_=psum[:])
                nc.sync.dma_start(
                    out=cin.ap()[st * 128:(st + 1) * 128, :], in_=xp_sb[:]
                )

            # ---- pipelined sub-AllGathers ----
            for p in range(NSUB):
                nc.gpsimd.collective_compute(
                    "AllGather",
                    mybir.AluOpType.bypass,
                    ins=[cin.ap()[p * SUB_ROWS:(p + 1) * SUB_ROWS, :].opt()],
                    outs=[cout.ap()[p * NC * SUB_ROWS:(p + 1) * NC * SUB_ROWS, :].opt()],
                    replica_groups=[list(range(NC))],
                )

            # ---- phase 2: replicated recurrence ----
            hts = [
                state_pool.tile([128, KT * 64], BF16, tag=f"ht{i}", name=f"ht_{i}")
                for i in range(2)
            ]
            chain = state_pool.tile([128, 2048], F32, tag="chain")
            nc.vector.memset(hts[0][:], 0)
            nc.vector.memset(chain[:, 1536:2048], 0)  # s = 2c = 0

            for t in range(n_steps):
                ht_cur = hts[t % 2]
                ht_nxt = hts[(t + 1) % 2]

                # x_proj rows for this step -> [128=(q,b), 2048]
                xpt = xpt_pool.tile([128, 2048], BF16)
                r0 = cout_row(t)
                src = cout.ap()[r0:r0 + 64, :].rearrange(
                    "b (ty q u) -> q b ty u", q=2, u=512
                )
                for q in range(2):
                    nc.sync.dma_start(
                        out=xpt[q * 64:(q + 1) * 64, :].rearrange(
                            "b (ty u) -> b ty u", u=512
                        ),
                        in_=src[q],
                    )

                gps = [
                    ps2_pool.tile([128, 512], F32, tag=f"g{ty}", name=f"gps_{ty}")
                    for ty in range(4)
                ]
                t_o = work_pool.tile([128, 512], F32, tag="to")
                av = work_pool.tile([128, 512], F32, tag="av")
                bv = work_pool.tile([128, 512], F32, tag="bv")
                tc_t = work_pool.tile([128, 512], F32, tag="tc")

                def bank(ty):
                    nc.tensor.matmul(
                        gps[ty][:],
                        lhsT=id128_sb[:],
                        rhs=xpt[:, ty * 512:(ty + 1) * 512],
                        start=True,
                        stop=False,
                    )
                    for k in range(KT):
                        hk = ht_cur[:, k * 64:(k + 1) * 64]
                        nc.tensor.matmul(
                            gps[ty][0:64, :],
                            lhsT=hk,
                            rhs=whh_sb[:, k * G + ty * 1024: k * G + ty * 1024 + 512],
                            start=False,
                            stop=False,
                            tile_position=(0, 0),
                        )
                        nc.tensor.matmul(
                            gps[ty][64:128, :],
                            lhsT=hk,
                            rhs=whh_sb[:, k * G + ty * 1024 + 512: k * G + (ty + 1) * 1024],
                            start=False,
                            stop=(k == KT - 1),
                            tile_position=(0, 64),
                        )

                # banks: i, f (A after), g (B, s', tc after), o
                bank(0)
                nc.scalar.activation(chain[:, 0:512], gps[0][:], Tanh)
                bank(1)
                nc.scalar.activation(chain[:, 512:1024], gps[1][:], Tanh)
                nc.vector.scalar_tensor_tensor(
                    out=av[:], in0=chain[:, 512:1024], scalar=1.0,
                    in1=chain[:, 1536:2048], op0=ADD, op1=MULT,
                )
                bank(2)
                nc.scalar.activation(chain[:, 1024:1536], gps[2][:], Tanh)
                nc.vector.scalar_tensor_tensor(
                    out=bv[:], in0=chain[:, 0:512], scalar=1.0,
                    in1=chain[:, 1024:1536], op0=ADD, op1=MULT,
                )
                nc.vector.scalar_tensor_tensor(
                    out=chain[:, 1536:2048], in0=av[:], scalar=0.5,
                    in1=bv[:], op0=MULT, op1=ADD,
                )
                nc.scalar.activation(tc_t[:], chain[:, 1536:2048], Tanh, scale=0.5)
                bank(3)
                nc.scalar.activation(t_o[:], gps[3][:], Tanh)

                if t < n_steps - 1:
                    htn = work_pool.tile([128, 512], BF16, tag="htn")
                    for k in range(KT):
                        q, blk = k // 4, k % 4
                        sl = slice(blk * 128, (blk + 1) * 128)
                        rows = slice(q * 64, (q + 1) * 64)
                        nc.vector.scalar_tensor_tensor(
                            out=htn[rows, sl], in0=t_o[rows, sl], scalar=1.0,
                            in1=tc_t[rows, sl], op0=ADD, op1=MULT,
                        )
                        pst = pstr_pool.tile([128, 64], BF16, tag=f"tr{k % 2}", name=f"pst_{k}")
                        nc.tensor.transpose(pst[:], htn[rows, sl], id_sb[rows, :])
                        nc.vector.tensor_copy(
                            ht_nxt[:, k * 64:(k + 1) * 64], pst[:]
                        )
                else:
                    hfin = work_pool.tile([128, 512], F32, tag="hfin")
                    nc.vector.scalar_tensor_tensor(
                        out=hfin[:], in0=t_o[:], scalar=1.0, in1=tc_t[:],
                        op0=ADD, op1=MULT,
                    )
                    for q in range(2):
                        nc.sync.dma_start(
                            out=h_out.ap()[:, q * 512:(q + 1) * 512],
                            in_=hfin[q * 64:(q + 1) * 64, :],
                        )

    nc.compile()
    return nc


_CACHED = {}


def _get_nc(n_steps=S):
    if n_steps not in _CACHED:
        _CACHED[n_steps] = build(n_steps)
    return _CACHED[n_steps]


def prepare_inputs(x, W_ih, W_hh, b_ih, b_hh):
    """Host-side layout prep. Returns per-core input maps."""
    x = np.asarray(x, dtype=np.float32)
    W_ih = np.asarray(W_ih, dtype=np.float32)
    W_hh = np.asarray(W_hh, dtype=np.float32)
    b = np.asarray(b_ih, dtype=np.float32) + np.asarray(b_hh, dtype=np.float32)

    # gate-row scaling: i,f,o rows *0.5 (sigmoid fold); W_hh all rows *0.5 (ht=2h)
    col = np.ones(G, dtype=np.float32)
    col[0:H] = 0.5
    col[H:2 * H] = 0.5
    col[3 * H:4 * H] = 0.5
    Wih_s = W_ih * col[:, None]
    Whh_s = W_hh * (0.5 * col[:, None])
    b_s = b * col

    wihT = np.ascontiguousarray(Wih_s.T).astype(bfloat16)
    whhT = np.ascontiguousarray(Whh_s.T).astype(bfloat16)
    bias = b_s.reshape(1, G).astype(bfloat16)
    xT_full = np.ascontiguousarray(x.reshape(SB, H).T).astype(bfloat16)
    eye64 = np.eye(64, dtype=np.float32)
    ident = np.vstack([eye64, eye64]).astype(bfloat16)
    id128_np = np.eye(128, dtype=np.float32).astype(bfloat16)
    ones = np.ones((1, 128), dtype=np.float32).astype(bfloat16)

    in_maps = []
    for j in range(NC):
        in_maps.append({
            "xT": np.ascontiguousarray(xT_full[:, j * SB_CORE:(j + 1) * SB_CORE]),
            "wihT": wihT,
            "whhT": whhT,
            "bias": bias,
            "ident": ident,
            "id128": id128_np,
            "ones": ones,
        })
    return in_maps


def kernel(x, W_ih, W_hh, b_ih, b_hh, trace=False, n_steps=S):
    nc = _get_nc(n_steps)
    in_maps = prepare_inputs(x, W_ih, W_hh, b_ih, b_hh)
    res = run_bass_kernel_spmd(nc, in_maps, core_ids=list(range(NC)), trace=trace)
    h_tilde = res.results[0]["h_last"]
    out = (0.5 * h_tilde).astype(np.float32)
    if trace:
        kernel.last_exec_time_ns = res.exec_time_ns
    return out
